# revision 7
# baseline (speedup 1.0000x reference)
"""Bahdanau additive attention kernel for Trainium2, data-parallel over 8 NeuronCores.

reference:
    W_hidden = hidden @ W_w.T + W_b                      # [A]
    U_encode = encoder_outputs @ U_w.T + U_b             # [S, A]
    poly_tanh = tanh(U_encode + W_hidden)                # [S, A]
    scores = poly_tanh @ V_w.T + V_b                     # [S, 1]
    attn = softmax(scores, axis=0)
    context = attn.T @ encoder_outputs                   # -> [1, 1, H]

Sharding: encoder_outputs split along S across the 8 cores; the small
U/W/V weights are replicated.  Each core emits an unnormalized partial
context sum(p_s * enc[s, :]) and partial denominator sum(p_s) with
p_s = exp(score_s - c) for a host-chosen constant shift c (softmax is
shift-invariant, so V_b is dropped and c only guards against overflow).
The host reduces the 8 partials — the one "all-reduce" this graph needs.

On-core layout (per core, S_loc = 1024):
  UE[a, s] = sum_h U_w[a, h] * enc[s, h]: PE matmuls with K=h on the
  partition dim.  lhsT = U_wT tiles [128h, 128a] (streamed), rhs = encT
  tiles [128h, 512s] (resident).  float32r operands: 1 cycle/row at
  N>=256 with ~16-bit mantissa.  Bias+tanh fused on ScalarE (bias is
  per-partition since a sits on partitions).  Scores: M=1 matmuls
  (lhsT = V column, rhs = tanh tile) accumulated over a-tiles into a
  [1, 512] PSUM row.  exp on ScalarE.  p broadcast across partitions by
  a K=1 ones-matmul.  Context: 32 fused multiply-reduce ops on VectorE
  against the resident encT (encT[:, kt*S_loc + s] = enc[s, kt*128+p],
  i.e. already h-on-partitions / s-on-free, exactly what a free-axis
  weighted reduction needs).
"""

import os
import sys
from contextlib import ExitStack

if "/opt/trn_rl_repo" not in sys.path:
    sys.path.insert(0, "/opt/trn_rl_repo")

import numpy as np

S, H, A, NCORES = 8192, 4096, 1024, 8
S_LOC = S // NCORES  # 1024
P = 128
KT = H // P  # 32 k-tiles (contraction over h)
AT = A // P  # 8 a-tiles
NSC = 2  # s-chunks of 512 per core
SCW = S_LOC // NSC  # 512

MODE = os.environ.get("NN_ATTN_MODE", "f32r")  # f32r | f32
TRACE = bool(int(os.environ.get("NN_ATTN_TRACE", "0")))
STAGE = int(os.environ.get("NN_ATTN_STAGE", "4"))

LAST_EXEC_NS = None
LAST_RESULTS = None

_COMPILED = {}


def _build(mode, stage=None):
    if stage is None:
        stage = STAGE
    import concourse.bacc as bacc
    import concourse.mybir as mybir
    import concourse.tile as tile

    dt = mybir.dt
    mm_dt = dt.float32r if mode == "f32r" else dt.float32

    nc = bacc.Bacc("TRN2", target_bir_lowering=False, debug=False)

    enc_t = nc.dram_tensor("enc_t", [KT, P, S_LOC], mm_dt, kind="ExternalInput").ap()
    u_t = nc.dram_tensor("u_t", [AT, P, KT, P], mm_dt, kind="ExternalInput").ap()
    bias_a = nc.dram_tensor("bias_a", [AT, P], dt.float32, kind="ExternalInput").ap()
    v_w = nc.dram_tensor("v_w", [AT, P], mm_dt, kind="ExternalInput").ap()
    neg_c = nc.dram_tensor("neg_c", [1, 1], dt.float32, kind="ExternalInput").ap()
    ctx_o = nc.dram_tensor("ctx_out", [H], dt.float32, kind="ExternalOutput").ap()
    l_o = nc.dram_tensor("l_out", [1, 1], dt.float32, kind="ExternalOutput").ap()

    with tile.TileContext(nc) as tc, ExitStack() as ctx:
        const = ctx.enter_context(tc.tile_pool(name="const", bufs=1))
        enc_pool = ctx.enter_context(tc.tile_pool(name="enc", bufs=1))
        u_pool = ctx.enter_context(tc.tile_pool(name="u", bufs=3))
        t_pool = ctx.enter_context(tc.tile_pool(name="t", bufs=3))
        out_pool = ctx.enter_context(tc.tile_pool(name="out", bufs=1))
        ps_ue = ctx.enter_context(tc.tile_pool(name="ps_ue", bufs=2, space="PSUM"))
        ps_sc = ctx.enter_context(tc.tile_pool(name="ps_sc", bufs=2, space="PSUM"))
        ps_pb = ctx.enter_context(tc.tile_pool(name="ps_pb", bufs=2, space="PSUM"))

        # constants / small inputs
        bias_sb = const.tile([P, AT], dt.float32)
        nc.sync.dma_start(bias_sb[:], bias_a.rearrange("at p -> p at"))
        v_sb = const.tile([P, AT], mm_dt)
        nc.sync.dma_start(v_sb[:], v_w.rearrange("at p -> p at"))
        negc_sb = const.tile([1, 1], dt.float32)
        nc.sync.dma_start(negc_sb[:], neg_c[:])
        ones_sb = const.tile([1, P], dt.float32)
        nc.vector.memset(ones_sb[:], 1.0)

        # resident encT: [128, KT*S_LOC]; free index = kt*S_LOC + s
        enc_sb = enc_pool.tile([P, KT * S_LOC], mm_dt)
        n_chunk = 4
        kt_per = KT // n_chunk
        for c in range(n_chunk):
            nc.sync.dma_start(
                enc_sb[:, c * kt_per * S_LOC : (c + 1) * kt_per * S_LOC],
                enc_t[c * kt_per : (c + 1) * kt_per].rearrange("kt p s -> p kt s"),
            )

        # score accumulators [1, 512] per s-chunk
        psc = [
            ps_sc.tile([1, SCW], dt.float32, tag="psc", name=f"psc{i}")
            for i in range(NSC)
        ]
        if stage < 2:
            for t in psc:
                nc.vector.memset(t[:], 1.0)

        for a in range(AT):
            # stream this a-tile's U_wT slab in two halves of 16 k-tiles
            u_half = []
            for hhalf in range(2):
                uh = u_pool.tile([P, (KT // 2) * P], mm_dt, tag="u")
                nc.sync.dma_start(
                    uh[:], u_t[a][:, hhalf * (KT // 2) : (hhalf + 1) * (KT // 2), :]
                )
                u_half.append(uh)
            for sc in range(NSC):
                pue = ps_ue.tile([P, SCW], dt.float32, tag="ue")
                for kt in range(KT):
                    uh = u_half[kt // (KT // 2)]
                    ki = kt % (KT // 2)
                    nc.tensor.matmul(
                        pue[:],
                        uh[:, ki * P : (ki + 1) * P],
                        enc_sb[:, kt * S_LOC + sc * SCW : kt * S_LOC + sc * SCW + SCW],
                        start=(kt == 0),
                        stop=(kt == KT - 1),
                    )
                # T = tanh(UE + bias_a) ; bias is per-partition (a on partitions)
                t_sb = t_pool.tile([P, SCW], mm_dt, tag="t")
                nc.scalar.activation(
                    t_sb[:], pue[:], mybir.ActivationFunctionType.Tanh,
                    bias=bias_sb[:, a : a + 1],
                )
                # scores[sc] += V[a-tile].T @ T   -> [1, 512]
                if stage >= 2:
                    nc.tensor.matmul(
                        psc[sc][:],
                        v_sb[:, a : a + 1],
                        t_sb[:],
                        start=(a == 0),
                        stop=(a == AT - 1),
                    )
                elif a == AT - 1:
                    # keep t_sb consumed so outputs exist
                    nc.vector.tensor_copy(psc[sc][:], t_sb[0:1, :])

        # p = exp(scores - c) on partition 0: [1, S_LOC]
        p_row = const.tile([1, S_LOC], dt.float32)
        if stage >= 3:
            for sc in range(NSC):
                nc.scalar.activation(
                    p_row[:, sc * SCW : (sc + 1) * SCW], psc[sc][:],
                    mybir.ActivationFunctionType.Exp, bias=negc_sb[0:1, 0:1],
                )
        else:
            nc.vector.memset(p_row[:], 1.0)
            nc.vector.tensor_copy(p_row[0:1, 0:1], psc[0][0:1, 0:1])
        # partial denominator l = sum(p)
        l_sb = out_pool.tile([1, 1], dt.float32)
        nc.vector.reduce_sum(l_sb[:], p_row[:], axis=mybir.AxisListType.X)
        nc.sync.dma_start(l_o[:], l_sb[:])

        # broadcast p across partitions: p_bc[p, s] = p[s] via K=1 ones matmul
        p_bc = const.tile([P, S_LOC], dt.float32)
        if stage >= 4:
            for sc in range(NSC):
                pb = ps_pb.tile([P, SCW], dt.float32, tag="pb")
                nc.tensor.matmul(
                    pb[:], ones_sb[:], p_row[0:1, sc * SCW : (sc + 1) * SCW],
                    start=True, stop=True,
                )
                nc.vector.tensor_copy(p_bc[:, sc * SCW : (sc + 1) * SCW], pb[:])
        else:
            nc.vector.memset(p_bc[:], 1.0)
            nc.vector.tensor_copy(p_bc[0:1, 0:S_LOC], p_row[0:1, :])

        # context: ctx[kt*128+p] = sum_s enc_sb[p, kt*S_LOC+s] * p_bc[p, s]
        ctx_sb = out_pool.tile([P, KT], dt.float32)
        scratch = out_pool.tile([P, S_LOC], dt.float32)
        for kt in range(KT):
            nc.vector.tensor_mul(
                scratch[:],
                enc_sb[:, kt * S_LOC : (kt + 1) * S_LOC].bitcast(dt.float32),
                p_bc[:],
            )
            nc.vector.reduce_sum(
                ctx_sb[:, kt : kt + 1], scratch[:], axis=mybir.AxisListType.X
            )
        nc.sync.dma_start(ctx_o.rearrange("(kt p) -> p kt", p=P), ctx_sb[:])

    nc.compile()
    return nc


def _get_nc(mode):
    if mode not in _COMPILED:
        _COMPILED[mode] = _build(mode)
    return _COMPILED[mode]


def kernel(**inputs):
    global LAST_EXEC_NS, LAST_RESULTS
    from concourse.bass_utils import run_bass_kernel_spmd

    enc = np.ascontiguousarray(np.asarray(inputs["encoder_outputs"], dtype=np.float32))
    hidden = np.asarray(inputs["hidden"], dtype=np.float32)
    U_w = np.asarray(inputs["U_w"], dtype=np.float32)
    U_b = np.asarray(inputs["U_b"], dtype=np.float32)
    W_w = np.asarray(inputs["W_w"], dtype=np.float32)
    W_b = np.asarray(inputs["W_b"], dtype=np.float32)
    V_w = np.asarray(inputs["V_w"], dtype=np.float32)
    V_b = np.asarray(inputs["V_b"], dtype=np.float32)

    bias_full = (U_b + W_b + W_w @ hidden).astype(np.float32)  # [A]
    U_wT = np.ascontiguousarray(U_w.T)  # [H, A]
    u_t = np.ascontiguousarray(
        U_wT.reshape(KT, P, AT, P).transpose(2, 1, 0, 3)
    )  # [AT, 128, KT, 128]
    bias_t = np.ascontiguousarray(bias_full.reshape(AT, P))
    v_t = np.ascontiguousarray(V_w.reshape(AT, P))
    # softmax shift: scores are bounded by sum|V_w| (tanh in [-1,1]); only
    # shift when the bound could overflow exp in fp32.
    c = float(max(0.0, np.abs(V_w).sum() + abs(float(V_b[0])) - 30.0))
    negc = np.full((1, 1), -c, dtype=np.float32)

    in_maps = []
    for i in range(NCORES):
        shard = enc[i * S_LOC : (i + 1) * S_LOC]  # [S_LOC, H]
        enc_t_i = np.ascontiguousarray(shard.T).reshape(KT, P, S_LOC)
        in_maps.append(
            {
                "enc_t": enc_t_i,
                "u_t": u_t,
                "bias_a": bias_t,
                "v_w": v_t,
                "neg_c": negc,
            }
        )

    nc = _get_nc(MODE)
    res = run_bass_kernel_spmd(nc, in_maps, list(range(NCORES)), trace=TRACE)
    LAST_EXEC_NS = res.exec_time_ns
    LAST_RESULTS = res

    ctx = np.zeros(H, dtype=np.float64)
    l = 0.0
    for i in range(NCORES):
        ctx += res.results[i]["ctx_out"].astype(np.float64)
        l += float(res.results[i]["l_out"][0, 0])
    out = (ctx / l).astype(np.float32).reshape(1, 1, H)
    return out
